# revision 26
# baseline (speedup 1.0000x reference)
"""Cross-attention Trainium2 kernel (nn_CrossAttention_24575802868332).

Sharding v3 (tensor-parallel heads): 8 cores; core c handles batch
b = c//4 and head pair hp = c%4 (heads 2hp, 2hp+1) for ALL 4096
queries.  K/V/Q projections are computed only for the core's own two
heads (no replication).  The output projection produces a partial
(2-head) contribution for all 4096 rows; a chunked ReduceScatter over
each batch's 4-core group sums the head contributions, leaving each
core with 1024 rows (its group-rank quarter of each 1024-row q-group),
to which the bias is added before the final store.

Per-core phases (bf16 operands, fp32 PSUM accumulation):
  P1: K/V projection for the full context (M=4096), pair-packed:
      kt[128, 4096] (head-even dims on partitions 0:64, odd on 64:128),
      vt[128, 32 m-subtiles, 2 heads, 65] (col 64 = ones -> softmax l).
  P2: per q-group of 1024 rows (4 groups):
      Q proj -> qt[128, 1024]; per 512-q chunk: 32 m-subtile sweeps of
      S^T pair matmuls (K=64, row-packed) -> exp(0.125 S) on ScalarE
      (bf16) -> O^T accumulated in PSUM across ALL 32 subtiles;
      normalize via reciprocal of the l row + ones outer-product;
      partial out-proj (inner=128) -> DRAM; ReduceScatter the group's
      [1024, 1024] bf16 partial over the 4-core group (overlaps next
      group's compute); add bias, store 256 final rows per group.
"""

import os
import sys

sys.path.insert(0, "/opt/trn_rl_repo")

from contextlib import ExitStack

import numpy as np

import concourse.bass as bass
import concourse.tile as tile
from concourse import bacc, mybir

F32 = mybir.dt.float32
F32R = mybir.dt.float32r
BF16 = mybir.dt.bfloat16
AF = mybir.ActivationFunctionType

# Problem constants (hardcoded per contract)
B, N, M = 2, 4096, 4096
DQ, DC, INNER = 1024, 768, 512
H, D = 8, 64
NCORES = 8
GROUPS = [[0, 1, 2, 3], [4, 5, 6, 7]]
NG = 4  # q-groups per core
QG = N // NG  # 1024 q rows per group
KQ = DQ // 128  # 8 k-chunks for q proj
KC = DC // 128  # 6 k-chunks for k/v proj
MS = M // 128  # 32 m-subtiles
MBLK = 512  # ctx projection chunk
NBLK = M // MBLK  # 8


def build_nc():
    nc = bacc.Bacc(
        "TRN2",
        target_bir_lowering=False,
        debug=False,
        enable_asserts=False,
        num_devices=NCORES,
    )
    xT = nc.dram_tensor("xT", [DQ, N], BF16, kind="ExternalInput").ap()
    ctxT = nc.dram_tensor("ctxT", [DC, M], BF16, kind="ExternalInput").ap()
    wq = nc.dram_tensor("wq", [DQ, 128], BF16, kind="ExternalInput").ap()
    wk = nc.dram_tensor("wk", [DC, 128], BF16, kind="ExternalInput").ap()
    wv = nc.dram_tensor("wv", [DC, 128], BF16, kind="ExternalInput").ap()
    wo = nc.dram_tensor("wo", [128, DQ], BF16, kind="ExternalInput").ap()
    bo = nc.dram_tensor("bo", [1, DQ], BF16, kind="ExternalInput").ap()
    ones_d = nc.dram_tensor("ones_d", [1, 128], F32R, kind="ExternalInput").ap()
    ones_b = nc.dram_tensor("ones_b", [1, 128], BF16, kind="ExternalInput").ap()
    out = nc.dram_tensor("out", [NG * 2, 128, DQ], BF16, kind="ExternalOutput").ap()

    with tile.TileContext(nc) as tc:
        _emit(nc, tc, xT, ctxT, wq, wk, wv, wo, bo, ones_d, ones_b, out)
    nc.compile()
    return nc


def _emit(nc, tc, xT, ctxT, wq, wk, wv, wo, bo, ones_d, ones_b, out):
    with ExitStack() as ctx:
        consts = ctx.enter_context(tc.tile_pool(name="consts", bufs=1))
        # ---- constants ----
        wq_sb = consts.tile([128, KQ, 128], BF16, tag="wq")
        nc.sync.dma_start(out=wq_sb, in_=wq.rearrange("(k p) n -> p k n", p=128))
        wk_sb = consts.tile([128, KC, 128], BF16, tag="wk")
        nc.sync.dma_start(out=wk_sb, in_=wk.rearrange("(k p) n -> p k n", p=128))
        wv_sb = consts.tile([128, KC, 128], BF16, tag="wv")
        nc.sync.dma_start(out=wv_sb, in_=wv.rearrange("(k p) n -> p k n", p=128))
        wo_sb = consts.tile([128, DQ], BF16, tag="wo")
        nc.sync.dma_start(out=wo_sb, in_=wo)
        ones_row = consts.tile([1, 128], F32R, tag="ones_row")
        nc.sync.dma_start(out=ones_row, in_=ones_d)
        # bias broadcast to all 128 partitions
        bias_sb = consts.tile([128, DQ], BF16, tag="bias")
        bias_bcast = bass.AP(tensor=bo.tensor, offset=0, ap=[[0, 128], [1, DQ]])
        nc.gpsimd.dma_start(out=bias_sb, in_=bias_bcast)
        # ones replicated to all 128 partitions (v_aug ones column source)
        ones_col = consts.tile([128, MS * 2], BF16, tag="ones_col")
        ones_bcast = bass.AP(
            tensor=ones_b.tensor, offset=0, ap=[[0, 128], [1, MS * 2]]
        )
        nc.gpsimd.dma_start(out=ones_col, in_=ones_bcast)

        # persistent K/V for the core's two heads
        acc = ctx.enter_context(tc.tile_pool(name="acc", bufs=1))
        kt = acc.tile([128, M], BF16, tag="kt")  # pair-packed k^T
        vt = acc.tile([128, MS, 2, 65], BF16, tag="vt")
        nc.vector.tensor_copy(
            vt[:, :, :, 64:65],
            ones_col[:].rearrange("p (a h o) -> p a h o", a=MS, h=2),
        )

        xTr = xT.rearrange("(k p) q -> p k q", p=128)
        ctxTr = ctxT.rearrange("(k p) m -> p k m", p=128)
        # all of x^T stays SBUF-resident: group-boundary Q projections then
        # never wait on DMA (in-flight collectives starve the DMA queues)
        xt_all = acc.tile([128, KQ, N], BF16, tag="xt_all")

        with ExitStack() as actx:
            qt_pool = actx.enter_context(tc.tile_pool(name="qt", bufs=2))
            aux_ps = actx.enter_context(
                tc.tile_pool(name="aux", bufs=2, space="PSUM")
            )

            def emit_qproj(qg):
                """Q projection for one 1024-row group (tensor block)."""
                q0 = qg * QG
                qt = qt_pool.tile([128, QG], BF16, tag="qt", name=f"qt{qg}")
                qt_ps = [
                    aux_ps.tile([128, 512], F32, tag="aux", name=f"qps{qg}_{qc}")
                    for qc in range(2)
                ]
                for kc in range(KQ):
                    for qc in range(2):
                        nc.tensor.matmul(
                            qt_ps[qc],
                            wq_sb[:, kc, :],
                            xt_all[:, kc, q0 + qc * 512 : q0 + (qc + 1) * 512],
                            start=(kc == 0),
                            stop=(kc == KQ - 1),
                        )
                for qc in range(2):
                    nc.vector.tensor_copy(
                        qt[:, qc * 512 : (qc + 1) * 512], qt_ps[qc]
                    )
                return qt

            # ---- P1: K/V projection over all M ----
            with tc.tile_pool(name="cx", bufs=4) as cx_pool, tc.tile_pool(
                name="kvps", bufs=2, space="PSUM"
            ) as kvps:
                for mc in range(NBLK):
                    cx = cx_pool.tile(
                        [128, KC, MBLK], BF16, tag="cx", name=f"cx{mc}"
                    )
                    m0 = mc * MBLK
                    nc.sync.dma_start(out=cx, in_=ctxTr[:, :, m0 : m0 + MBLK])
                    kp = kvps.tile([128, MBLK], F32, tag="kvps", name=f"kp{mc}")
                    for kc in range(KC):
                        nc.tensor.matmul(
                            kp,
                            wk_sb[:, kc, :],
                            cx[:, kc, :],
                            start=(kc == 0),
                            stop=(kc == KC - 1),
                        )
                    nc.vector.tensor_copy(kt[:, m0 : m0 + MBLK], kp)
                    vp = kvps.tile([128, MBLK], F32, tag="kvps", name=f"vp{mc}")
                    for ms in range(4):
                        for kc in range(KC):
                            nc.tensor.matmul(
                                vp[:, ms * 128 : (ms + 1) * 128],
                                cx[:, kc, ms * 128 : (ms + 1) * 128],
                                wv_sb[:, kc, :],
                                start=(kc == 0),
                                stop=(kc == KC - 1),
                            )
                    nc.vector.tensor_copy(
                        vt[:, mc * 4 : (mc + 1) * 4, :, 0:64],
                        vp[:].rearrange("p (a h d) -> p a h d", a=4, h=2),
                    )

            # prefetch all of x^T (group 0 first); then Q proj for group 0
            for qg in range(NG):
                q0 = qg * QG
                for kc in range(KQ):
                    nc.sync.dma_start(
                        out=xt_all[:, kc, q0 : q0 + QG],
                        in_=xTr[:, kc, q0 : q0 + QG],
                    )
            qt_cur = emit_qproj(0)

            # ---- P2: attention + per-512-row tail, chunked ReduceScatter ----
            osb_pool = actx.enter_context(tc.tile_pool(name="osb", bufs=2))
            ko_pool = actx.enter_context(tc.tile_pool(name="ko", bufs=2))
            po_pool = actx.enter_context(tc.tile_pool(name="po", bufs=2))
            norm_pool = actx.enter_context(tc.tile_pool(name="norm", bufs=2))
            p_pool = actx.enter_context(tc.tile_pool(name="p", bufs=3))
            s_ps = actx.enter_context(
                tc.tile_pool(name="sps", bufs=2, space="PSUM")
            )
            o_ps = actx.enter_context(
                tc.tile_pool(name="ops", bufs=2, space="PSUM")
            )
            dram_in = actx.enter_context(
                tc.tile_pool(name="dpart", bufs=2, space="DRAM")
            )
            dram_out = actx.enter_context(
                tc.tile_pool(name="drs", bufs=2, space="DRAM")
            )

            def make_tail(qg, qc, o_sb):
                """Normalize + partial out-proj + RS + bias/store for one
                512-row chunk, as thunks to sprinkle into the next sweep.
                The post-collective DMAs ride the gpsimd queue so the RS
                wait never blocks the compute-feed (sync) DMA queue."""
                th = []
                ck = qg * 2 + qc
                recip2 = norm_pool.tile(
                    [2, 512], F32R, tag="recip2", name=f"r2_{ck}"
                )
                nc.sync.dma_start(
                    out=recip2, in_=o_sb[64:65, :, :].bitcast(F32R)
                )
                with nc.allow_low_precision(reason="1/l in fp32r is fine"):
                    nc.vector.reciprocal(recip2[:], recip2[:])
                recip_sb = norm_pool.tile(
                    [1, 2, 512], F32R, tag="recip", name=f"rsc_{ck}"
                )
                nc.sync.dma_start(out=recip_sb, in_=recip2[:, :])
                ko = ko_pool.tile([128, 512], BF16, tag="ko", name=f"ko{ck}")
                o_bf = ko_pool.tile(
                    [64, 2, 512], BF16, tag="obf", name=f"obf{ck}"
                )

                def t_norm(j):
                    bp = aux_ps.tile([64, 512], F32, tag="aux", name=f"bp{ck}_{j}")
                    nc.tensor.matmul(
                        bp,
                        ones_row[0:1, 0:64],
                        recip_sb[:, j, :],
                        start=True,
                        stop=True,
                    )
                    nc.vector.tensor_mul(o_bf[:, j, :], o_sb[0:64, j, :], bp)

                th.append(lambda: t_norm(0))
                th.append(lambda: t_norm(1))
                th.append(
                    lambda: nc.sync.dma_start(out=ko[0:64, :], in_=o_bf[:, 0, :])
                )
                th.append(
                    lambda: nc.sync.dma_start(out=ko[64:128, :], in_=o_bf[:, 1, :])
                )
                partial = dram_in.tile([512, DQ], BF16, tag="part", name=f"pt{ck}")

                def t_proj(ql):
                    po = po_pool.tile([128, DQ], BF16, tag="po", name=f"po{ck}_{ql}")
                    for nck in range(2):
                        pp = aux_ps.tile(
                            [128, 512], F32, tag="aux", name=f"pp{ck}_{ql}_{nck}"
                        )
                        nc.tensor.matmul(
                            pp,
                            ko[:, ql * 128 : (ql + 1) * 128],
                            wo_sb[:, nck * 512 : (nck + 1) * 512],
                            start=True,
                            stop=True,
                        )
                        # fold in bias/4 (summed to full bias by the RS)
                        nc.vector.tensor_add(
                            po[:, nck * 512 : (nck + 1) * 512],
                            pp,
                            bias_sb[:, nck * 512 : (nck + 1) * 512],
                        )
                    nc.sync.dma_start(
                        out=partial[ql * 128 : (ql + 1) * 128, :], in_=po
                    )

                for ql in range(4):
                    th.append(lambda ql=ql: t_proj(ql))

                def t_rs():
                    rs_out = dram_out.tile([128, DQ], BF16, tag="rs", name=f"rs{ck}")
                    nc.gpsimd.collective_compute(
                        "ReduceScatter",
                        mybir.AluOpType.add,
                        replica_groups=GROUPS,
                        ins=[partial[:].opt()],
                        outs=[rs_out[:].opt()],
                    )
                    nc.gpsimd.dma_start(out=out[ck, :, :], in_=rs_out[:, :])

                th.append(t_rs)
                return th

            pending = []
            for qg in range(NG):
                qt = qt_cur
                for qc in range(2):
                    # ---- attention sweep: O^T accumulates across all M ----
                    o_sb = osb_pool.tile(
                        [65, 2, 512], F32, tag="osb", name=f"osb{qg}_{qc}"
                    )
                    ops_e = o_ps.tile([65, 512], F32, tag="ops", name=f"oe{qg}_{qc}")
                    ops_o = o_ps.tile([65, 512], F32, tag="ops", name=f"oo{qg}_{qc}")
                    o_emits = []
                    # pop tails late: their DMA deps are satisfied by then and
                    # the RS fires near sweep end, clear of feed DMAs
                    sp0 = 18
                    per = (len(pending) + MS - sp0 - 2) // max(MS - sp0 - 1, 1)
                    for mt in range(MS):
                        sl = s_ps.tile(
                            [128, 1024], F32, tag="sps", name=f"sl{qg}_{qc}_{mt}"
                        )
                        nc.tensor.matmul(
                            sl[:, 0:512],
                            kt[0:64, mt * 128 : (mt + 1) * 128],
                            qt[0:64, qc * 512 : (qc + 1) * 512],
                            start=True,
                            stop=True,
                        )
                        nc.tensor.matmul(
                            sl[:, 512:1024],
                            kt[64:128, mt * 128 : (mt + 1) * 128],
                            qt[64:128, qc * 512 : (qc + 1) * 512],
                            start=True,
                            stop=True,
                        )
                        psl = p_pool.tile(
                            [128, 1024], BF16, tag="p", name=f"psl{qg}_{qc}_{mt}"
                        )
                        nc.scalar.activation(psl, sl, AF.Exp, scale=0.125)

                        def o_pair(mt=mt, psl=psl, ops_e=ops_e, ops_o=ops_o):
                            nc.tensor.matmul(
                                ops_e,
                                vt[:, mt, 0, :],
                                psl[:, 0:512],
                                start=(mt == 0),
                                stop=(mt == MS - 1),
                            )
                            nc.tensor.matmul(
                                ops_o,
                                vt[:, mt, 1, :],
                                psl[:, 512:1024],
                                start=(mt == 0),
                                stop=(mt == MS - 1),
                            )

                        o_emits.append(o_pair)
                        # software pipeline: O lags S by one slab
                        if mt >= 1:
                            o_emits.pop(0)()
                        # sprinkle previous chunk's tail into this sweep
                        if mt >= sp0:
                            for _ in range(per):
                                if pending:
                                    pending.pop(0)()
                    while o_emits:
                        o_emits.pop(0)()
                    for t in pending:  # leftovers
                        t()
                    pending = []
                    nc.vector.tensor_copy(o_sb[:, 0, :], ops_e)
                    nc.vector.tensor_copy(o_sb[:, 1, :], ops_o)
                    if qc == 1 and qg + 1 < NG:
                        qt_cur = emit_qproj(qg + 1)
                    pending = make_tail(qg, qc, o_sb)
            for t in pending:  # last chunk's tail
                t()


_NC_CACHE = None


def _get_nc():
    global _NC_CACHE
    if _NC_CACHE is None:
        _NC_CACHE = build_nc()
    return _NC_CACHE


def shard_inputs(x, context, Wq, Wk, Wv, Wo, bo):
    import ml_dtypes

    bf16 = ml_dtypes.bfloat16
    ones = np.ones((1, 128), np.float32)
    ones_b = np.ones((1, 128), bf16)
    # bias/4 per core: the 4-way ReduceScatter sum restores the full bias
    bo2 = np.ascontiguousarray(
        (np.asarray(bo, np.float32).reshape(1, DQ) * 0.25).astype(bf16)
    )
    Wq = np.asarray(Wq, np.float32)
    Wk = np.asarray(Wk, np.float32)
    Wv = np.asarray(Wv, np.float32)
    Wo = np.ascontiguousarray(np.asarray(Wo, np.float32))
    xT = [np.ascontiguousarray(x[b].T.astype(bf16)) for b in range(B)]
    ctxT = [np.ascontiguousarray(context[b].T.astype(bf16)) for b in range(B)]
    maps = []
    for c in range(NCORES):
        b = c // 4
        hp = c % 4
        maps.append(
            {
                "xT": xT[b],
                "ctxT": ctxT[b],
                "wq": np.ascontiguousarray(
                    Wq[:, hp * 128 : (hp + 1) * 128].astype(bf16)
                ),
                "wk": np.ascontiguousarray(
                    Wk[:, hp * 128 : (hp + 1) * 128].astype(bf16)
                ),
                "wv": np.ascontiguousarray(
                    Wv[:, hp * 128 : (hp + 1) * 128].astype(bf16)
                ),
                "wo": np.ascontiguousarray(
                    Wo[hp * 128 : (hp + 1) * 128, :].astype(bf16)
                ),
                "bo": bo2,
                "ones_d": ones,
                "ones_b": ones_b,
            }
        )
    return maps


def kernel(x, context, Wq, Wk, Wv, Wo, bo):
    from concourse.bass_utils import run_bass_kernel_spmd

    x = np.asarray(x, np.float32)
    context = np.asarray(context, np.float32)
    maps = shard_inputs(x, context, Wq, Wk, Wv, Wo, bo)
    nc = _get_nc()
    trace = os.environ.get("KERNEL_TRACE", "0") == "1"
    res = run_bass_kernel_spmd(
        nc, maps, core_ids=list(range(NCORES)), trace=trace
    )
    full = np.empty((B, N, DQ), np.float32)
    for c in range(NCORES):
        b = c // 4
        r = c % 4
        o = np.asarray(res.results[c]["out"], np.float32)  # [8, 128, DQ] bf16
        for ck in range(NG * 2):
            r0 = ck * 512 + r * 128
            full[b, r0 : r0 + 128, :] = o[ck]
    if trace:
        kernel.last_exec_time_ns = res.exec_time_ns
        kernel.last_profile_json = res.profile_json
        kernel.last_trace_path = (
            res.instructions_and_trace[1] if res.instructions_and_trace else None
        )
    return full
